# revision 1
# baseline (speedup 1.0000x reference)
"""Trainium2 Bass kernel for CausalGraphNetwork.

Computes, for x = step_sequence [B=2, N=512, H=256]:
    h  = relu(x @ W_gc1.T + b_gc1)
    f  = relu(h @ W_gc2.T + b_gc2)
    a  = f @ Wa.T + b_ep1    (Wa = W_ep1[:, :H])
    c  = f @ Wb.T            (Wb = W_ep1[:, H:])
    e[b,i,j,:] = relu(a[b,i,:] + c[b,j,:])
    scores = sigmoid(e @ w_ep2 + b_ep2) * strict_lower_mask

Strategy (8 NeuronCores, SPMD single program):
  - Core d owns batch d//4, rows i = 4k + (d%4), k = 0..127 (interleaved
    so the causal work profile is identical on every core).
  - Upstream (h/f/c/a) runs in fp8_e4m3 with DoubleRow matmuls: x/h/f
    are packed [128, 2, cols] (two 128-partition k-tiles), weights
    [128, 2, 128], contracting all 256 inputs in one matmul at 2 fp8
    k-columns/cycle.  Weights are pre-scaled x16 on the host to clear
    the fp8 denormal floor; each epilogue applies scale=1/16.
  - Pairwise stage in bf16: per (row k, h-chunk c) one e tile
    e = relu(c_j + a_k) built by a tensor_scalar (DVE at 4x, or ACT /
    GPSIMD per a static makespan-balancing schedule), consumed by an
    M=1 matmul into PSUM partition 32*(k%4); two chunks accumulate.
  - PSUM: 6 banks (3 double-bank tiles) cycle through rows, 4 rows per
    bank; 2 banks for the upstream layers.  Every 8 rows one sigmoid +
    one DMA drain a double-bank.
  - The next rep's upstream is interleaved into the current rep's
    pairwise stream so e-gen engines never starve at rep boundaries.
"""

import ml_dtypes
import numpy as np

import bass_rust
import concourse.bass as bass
import concourse.mybir as mybir
import concourse.tile as tile
from concourse.bass_utils import run_bass_kernel_spmd
from concourse.vector_clock import ScopedClock

B, N, H = 2, 512, 256
NCORES = 8
R = 128  # rows per core
NT = N + R  # 640 token columns: 512 shared j-tokens + 128 own i-tokens
F32 = mybir.dt.float32
BF16 = mybir.dt.bfloat16
FP8 = mybir.dt.float8e4
AF = mybir.ActivationFunctionType
ALU = mybir.AluOpType
DR = mybir.MatmulPerfMode.DoubleRow

WS = 16.0                  # upstream weight pre-scale (fp8 denormal floor)
INV_WS = 1.0 / WS
E_SCALE = 8.0              # c/a pre-scale so fp8 e clears the denormal floor
W2_SCALE = 64.0            # w_ep2 pre-scale (fp8 denormal floor)
SIG_SCALE = 1.0 / (E_SCALE * W2_SCALE)

NB = 6                     # pairwise PSUM banks (3 tiles x 2 banks)

DVE_C = 1e9 / 0.96e9
ACT_C = 1e9 / 1.2e9
POOL_C = 1e9 / 1.2e9
PE_C = 1e9 / 2.4e9

# upstream stage trigger rows (pairwise row index at which the NEXT rep's
# upstream stage is emitted)
STAGE_AT = (("xt", 2), ("h", 12), ("f", 36), ("c", 64), ("a", 76))


def jbx(k: int) -> int:
    """Per-row causal compute extent (multiple of 8, >= 4k+4)."""
    return min(N, ((4 * k + 4) + 7) // 8 * 8)


def _mk_schedule():
    """Two-phase schedule: (1) global dtype choice per 4-row PSUM-bank
    group by steepest-descent on the engine makespan — fp8 groups use
    one zero-padded DoubleRow matmul per row (PE ~4x cheaper) at the
    price of 2x DVE e-gen (or unchanged ACT e-gen); (2) chunk-level
    engine assignment for bf16 groups via a prefix-balanced walk."""
    NG = R // 4
    up_act_total = (2 * (2 * 640 * ACT_C + 4 * 185)
                    + 1024 * ACT_C + 2 * 185 + 256 * ACT_C + 2 * 185)
    sig_total = sum(2 * min(N, 32 * (g + 1)) * ACT_C + 185
                    for g in range(NG // 2))
    pe0 = (2 * 723 + 578 + 145 + 12 * 256) * PE_C

    def g_cost(G, mode):
        ks = range(4 * G, 4 * G + 4)
        dve = act = pe = 0.0
        for k in ks:
            jb = jbx(k)
            if mode == "bf16_dve":
                dve += 2 * (jb / 4 + 58) * DVE_C
                pe += 2 * jb * PE_C
            elif mode == "fp8_dve":
                dve += 2 * (jb / 2 + 58) * DVE_C
                pe += (2 * (32 * (k % 4) + 1) + jb / 2 * 1.13) * PE_C
            elif mode == "fp8_act":
                act += 2 * (jb + 222) * ACT_C
                pe += (2 * (32 * (k % 4) + 1) + jb / 2 * 1.13) * PE_C
            elif mode == "bf16_act":
                act += 2 * (jb + 222) * ACT_C
                pe += 2 * jb * PE_C
        return dve, act, pe

    # quota enumeration: choose how many groups run in each mode and
    # which (by jb order), minimizing max(dve, act, pe) exactly
    fixed_act = up_act_total + sig_total
    costs = [dict((m, g_cost(G, m)) for m in
                  ("bf16_dve", "fp8_dve", "fp8_act", "bf16_act"))
             for G in range(NG)]
    best = None
    for n_af in range(0, 13):
        for n_df in range(0, 25):
            if n_af + n_df > NG:
                continue
            # spread conversions evenly across the group timeline
            modes_try = ["bf16_dve"] * NG
            used = set()
            for i in range(n_af):
                g = min(NG - 1, int((i + 0.5) * NG / n_af))
                while g in used:
                    g = (g + 1) % NG
                used.add(g)
                modes_try[g] = "fp8_act"
            for i in range(n_df):
                g = min(NG - 1, int((i + 0.25) * NG / n_df))
                while g in used:
                    g = (g + 1) % NG
                used.add(g)
                modes_try[g] = "fp8_dve"
            dve, act, pe = 0.0, fixed_act, pe0
            for G, m in enumerate(modes_try):
                d, a2, p = costs[G][m]
                dve += d; act += a2; pe += p
            mk = max(dve, act, pe)
            if best is None or mk < best[0]:
                best = (mk, list(modes_try), (dve, act, pe))
    modes = best[1]

    # phase 2: chunk-level engine assignment for bf16 groups via a
    # prefix-balanced walk (fp8 groups keep their uniform engine)
    up_act = {"h": 2 * 640 * ACT_C + 4 * 185, "f": 2 * 640 * ACT_C + 4 * 185,
              "c": 1024 * ACT_C + 2 * 185, "a": 256 * ACT_C + 2 * 185}
    trig = dict((at, st) for st, at in STAGE_AT)
    busy = {"dve": 0.0, "act": 0.0}
    sched = []
    for G in range(NG):
        m = modes[G]
        dt = "bf16" if m.startswith("bf16") else "fp8"
        geng = "dve" if m.endswith("dve") else "act"
        engs = []
        for k in range(4 * G, 4 * G + 4):
            if k in trig and trig[k] in up_act:
                busy["act"] += up_act[trig[k]]
            jb = jbx(k)
            if dt == "fp8":
                ecost = (2 * (jb / (2 if geng == "dve" else 1)
                              + (58 if geng == "dve" else 222))
                         * (DVE_C if geng == "dve" else ACT_C))
                busy[geng] += ecost
                engs.append((geng, geng))
                continue
            row = []
            for c in range(2):
                cand = None
                for eng, ecost in (("dve", (jb / 4 + 58) * DVE_C),
                                   ("act", (jb + 222) * ACT_C)):
                    sc = (max(busy[eng] + ecost,
                              *[v for e2, v in busy.items() if e2 != eng]),
                          busy[eng] + ecost)
                    if cand is None or sc < cand[0]:
                        cand = (sc, eng, ecost)
                _, eng, ecost = cand
                busy[eng] += ecost
                row.append(eng)
            engs.append(tuple(row))
        if G % 2 == 1:
            busy["act"] += 2 * min(N, 32 * (G // 2 + 1)) * ACT_C + 185
        sched.append((dt, engs))
    dve, act, pe = best[2]
    return sched, {"dve": dve, "act": act}, pe


SCHED, _LOADS, _PE_LOAD = _mk_schedule()


class _TC(tile.TileContext):
    """TileContext variant for a walrus build that only supports ONE sem
    wait per instruction: split multi-wait instructions by hoisting the
    extra waits onto NOPs inserted just before them."""

    MAXW = 1

    def _split_waits_in_list(self, insts):
        out = []
        for inst in insts:
            si = inst.sync_info
            waits = list(si.on_wait) if si is not None else []
            if len(waits) > self.MAXW:
                rest, keep = waits[: -self.MAXW], waits[-self.MAXW :]
                for i in range(0, len(rest), self.MAXW):
                    nop = mybir.InstNoOp(
                        name=self.nc.get_next_instruction_name(),
                        engine=inst.engine,
                        bass_nofuse=True,
                        sync_info=bass_rust.SyncInfo(
                            on_wait=rest[i : i + self.MAXW], on_update=[]
                        ),
                    )
                    out.append(nop)
                inst.sync_info = bass_rust.SyncInfo(
                    on_wait=keep, on_update=list(si.on_update)
                )
            out.append(inst)
        return out

    def _lower_ordered_insts(self, ordered):
        for bb_name in list(ordered.keys()):
            ordered[bb_name] = self._split_waits_in_list(ordered[bb_name])
        return super()._lower_ordered_insts(ordered)

    def _drain_and_barrier(self, tick_clock, wait_clock):
        drain_inst = self.nc.sync.drain()
        wait_clock.add_sem_waits(
            drain_inst.ins, ScopedClock({None: tick_clock.global_clock})
        )
        si = drain_inst.ins.sync_info
        waits = list(si.on_wait) if si is not None else []
        if len(waits) > self.MAXW:
            drain_inst.ins.sync_info = bass_rust.SyncInfo(
                on_wait=waits[: self.MAXW], on_update=list(si.on_update)
            )
            rest = waits[self.MAXW :]
            for i in range(0, len(rest), self.MAXW):
                nop = self.nc.sync.nop(nofuse=True, hint=f"dw{i}")
                nop.ins.sync_info = bass_rust.SyncInfo(
                    on_wait=rest[i : i + self.MAXW], on_update=[]
                )
        self.nc.all_engine_barrier()
        assert self.sems is not None
        popped = self.nc._tile_sem_poison_stack.pop()
        assert popped is self._sem_poison
        self.nc.clear_and_free_semaphores(list(self.sems.allocated().values()))
        self.nc.all_engine_barrier()


def _egen(nc, engine, out_ap, in_ap, bias_col):
    """e = relu(in + bias) with per-partition bias, on the chosen engine."""
    if engine == "dve":
        nc.vector.tensor_scalar(
            out=out_ap, in0=in_ap, scalar1=bias_col, scalar2=0.0,
            op0=ALU.add, op1=ALU.max,
        )
    elif engine == "pool":
        nc.gpsimd.tensor_scalar(
            out=out_ap, in0=in_ap, scalar1=bias_col, scalar2=0.0,
            op0=ALU.add, op1=ALU.max,
        )
    elif engine == "act":
        nc.scalar.activation(out_ap, in_ap, AF.Relu, bias=bias_col)
    else:
        raise ValueError(engine)


class _Ctx:
    """Holds pools, constants and psum tiles for the build."""


def _t2(t, width):
    """View a packed [128, 2*width] tile as [p, t, j]."""
    return t.rearrange("p (t j) -> p t j", t=2)


def _upstream_stage(nc, cx, st, stage):
    if stage == "xt":
        t = cx.wpool.tile([128, 2 * NT], FP8, name="xt8", tag="xt8")
        nc.sync.dma_start(t[:, :], cx.xt[:, :])
        st["xt"] = t
    elif stage in ("h", "f"):
        src = st["xt"] if stage == "h" else st["h"]
        wbase = "w1t" if stage == "h" else "w2t"
        bias = cx.b1t if stage == "h" else cx.b2t
        dst = cx.wpool.tile([128, 2 * NT], FP8, name=f"{stage}8",
                            tag=f"{stage}8")
        for oc in range(2):
            for (t0, tn) in ((0, 512), (512, R)):
                ps = cx.upsum[cx.up_rr % 2]
                cx.up_rr += 1
                nc.tensor.matmul(
                    ps[:, 0:tn],
                    lhsT=cx.wts[(wbase, oc)],
                    rhs=_t2(src, NT)[:, :, t0 : t0 + tn],
                    start=True, stop=True, perf_mode=DR,
                )
                nc.scalar.activation(
                    dst[:, oc * NT + t0 : oc * NT + t0 + tn], ps[:, 0:tn],
                    AF.Relu, bias=bias[:, oc : oc + 1], scale=INV_WS)
        st[stage] = dst
    elif stage == "c":
        st["cts"] = []
        for oc in range(2):
            ct = cx.wpool.tile([128, N], BF16, name=f"ct_{oc}",
                               tag=f"ct_{oc}")
            ps = cx.upsum[cx.up_rr % 2]
            cx.up_rr += 1
            nc.tensor.matmul(
                ps[:, 0:512],
                lhsT=cx.wts[("wbt", oc)],
                rhs=_t2(st["f"], NT)[:, :, 0:512],
                start=True, stop=True, perf_mode=DR,
            )
            nc.scalar.activation(ct[:, :], ps[:, 0:512], AF.Identity,
                                 scale=INV_WS)
            st["cts"].append(ct)
    elif stage == "a":
        st["ats"] = []
        for oc in range(2):
            at = cx.wpool.tile([128, R], F32, name=f"at_{oc}",
                               tag=f"at_{oc}")
            ps = cx.upsum[cx.up_rr % 2]
            cx.up_rr += 1
            nc.tensor.matmul(
                ps[:, 0:R],
                lhsT=cx.wts[("wat", oc)],
                rhs=_t2(st["f"], NT)[:, :, 512 : 512 + R],
                start=True, stop=True, perf_mode=DR,
            )
            nc.scalar.activation(at[:, :], ps[:, 0:R], AF.Identity,
                                 bias=cx.bep1t[:, oc : oc + 1], scale=INV_WS)
            st["ats"].append(at)
    else:
        raise ValueError(stage)


def _rep_body(nc, cx, cts, ats, next_st):
    """One rep: pairwise over 128 rows + drains, with the next rep's
    upstream stages interleaved at STAGE_AT trigger rows."""
    triggers = dict((at, s) for s, at in STAGE_AT) if next_st is not None \
        else {}
    for G in range(R // 4):
        dt, engs = SCHED[G]
        bank = G % NB
        tb, hb = bank // 2, bank % 2
        ps = cx.psum[tb]
        ks = list(range(4 * G, 4 * G + 4))
        for k in ks:
            if k in triggers:
                _upstream_stage(nc, cx, next_st, triggers[k])
        order = ks[::-1] if dt == "fp8" else ks
        for k in order:
            jb = jbx(k)
            u = k % 4
            if dt == "fp8":
                e8 = cx.e8pool.tile([128, 2 * N], FP8, name="e8", tag="e8",
                                    bufs=8)
                for c in range(2):
                    _egen(nc, engs[u][c], e8[:, c * N : c * N + jb],
                          cts[c][:, 0:jb], ats[c][:, k : k + 1])
                M = 32 * u + 1
                nc.tensor.matmul(
                    ps[0:M, 512 * hb : 512 * hb + jb],
                    lhsT=_t2(cx.wu8[u], 256)[:, :, 0:M],
                    rhs=_t2(e8, N)[:, :, 0:jb],
                    start=(u == 3), stop=(u == 0),
                    perf_mode=DR, skip_group_check=True,
                )
            else:
                for c in range(2):
                    eb = cx.ebpool.tile([128, N], BF16, name=f"eb{c}",
                                        tag=f"eb{c}", bufs=16)
                    _egen(nc, engs[u][c], eb[:, 0:jb], cts[c][:, 0:jb],
                          ats[c][:, k : k + 1])
                    nc.tensor.matmul(
                        ps[32 * u : 32 * u + 1, 512 * hb : 512 * hb + jb],
                        lhsT=cx.wep2t[:, c : c + 1],
                        rhs=eb[:, 0:jb],
                        start=(c == 0), stop=(c == 1),
                        tile_position=(0, 32 * u),
                    )
        if G % 2 == 1:
            g = G // 2
            jbb = min(N, 32 * (g + 1))
            ptile = cx.psum[g % 3]
            qin = _t2(ptile, N)[:, :, 0:jbb]
            sc = cx.scpool.tile([128, 2 * N], F32, name="sc", tag="sc",
                                bufs=4)
            qout = _t2(sc, N)[:, :, 0:jbb]
            nc.scalar.activation(qout, qin, AF.Sigmoid,
                                 bias=cx.bep2t[:, 0:1], scale=SIG_SCALE)
            r0 = 8 * g
            dst = cx.y[r0 : r0 + 8, 0:jbb].rearrange("(b u) j -> u b j", b=2)
            src = qout[0:128:32, :, :]
            nc.sync.dma_start(dst, src)


def build_nc(reps: int = 1) -> bass.Bass:
    nc = bass.Bass("TRN2", target_bir_lowering=False, debug=False)

    cx = _Ctx()
    cx.xt = nc.dram_tensor("xt", [128, 2 * NT], FP8, kind="ExternalInput")
    wdr = {}
    for nm in ("w1t", "w2t", "wat", "wbt"):
        wdr[nm] = nc.dram_tensor(nm, [128, 512], FP8, kind="ExternalInput")
    b1 = nc.dram_tensor("b1", [128, 2], F32, kind="ExternalInput")
    b2 = nc.dram_tensor("b2", [128, 2], F32, kind="ExternalInput")
    bep1 = nc.dram_tensor("bep1", [128, 2], F32, kind="ExternalInput")
    wep2 = nc.dram_tensor("wep2", [128, 2], BF16, kind="ExternalInput")
    wep2f8 = nc.dram_tensor("wep2f8", [128, 2048], FP8, kind="ExternalInput")
    bep2 = nc.dram_tensor("bep2", [128, 1], F32, kind="ExternalInput")
    cx.y = nc.dram_tensor("y", [R, N], F32, kind="ExternalOutput")

    with _TC(nc) as tc:
        with tc.tile_pool(name="const", bufs=1) as cpool, \
             tc.tile_pool(name="work", bufs=2) as wpool, \
             tc.tile_pool(name="ebpool", bufs=8) as ebpool, \
             tc.tile_pool(name="e8pool", bufs=8) as e8pool, \
             tc.tile_pool(name="scpool", bufs=4) as scpool:

            cx.wpool, cx.ebpool, cx.scpool = wpool, ebpool, scpool
            cx.e8pool = e8pool

            # ---- constants ----
            # upstream weights, packed [p, t(2), m_global(256)]: lhsT for
            # output chunk oc is the [:, :, oc*128:(oc+1)*128] slice
            cx.wts = {}
            qengs = [nc.sync, nc.scalar, nc.sync, nc.scalar]
            for qi, nm in enumerate(("w1t", "w2t", "wat", "wbt")):
                t = cpool.tile([128, 512], FP8, name=f"{nm}8")
                qengs[qi].dma_start(t[:, :], wdr[nm][:, :])
                for oc in range(2):
                    cx.wts[(nm, oc)] = t.rearrange(
                        "p (t m) -> p t m",
                        t=2)[:, :, oc * 128 : (oc + 1) * 128]
            cx.b1t = cpool.tile([128, 2], F32, name="b1t")
            nc.sync.dma_start(cx.b1t[:, :], b1[:, :])
            cx.b2t = cpool.tile([128, 2], F32, name="b2t")
            nc.sync.dma_start(cx.b2t[:, :], b2[:, :])
            cx.bep1t = cpool.tile([128, 2], F32, name="bep1t")
            nc.sync.dma_start(cx.bep1t[:, :], bep1[:, :])
            cx.wep2t = cpool.tile([128, 2], BF16, name="wep2t")
            nc.sync.dma_start(cx.wep2t[:, :], wep2[:, :])
            w8full = cpool.tile([128, 2048], FP8, name="w8full")
            nc.scalar.dma_start(w8full[:, :], wep2f8[:, :])
            cx.wu8 = [w8full[:, 512 * u : 512 * (u + 1)] for u in range(4)]
            cx.bep2t = cpool.tile([128, 1], F32, name="bep2t")
            nc.sync.dma_start(cx.bep2t[:, :], bep2[:, :])

            ppp = tc.alloc_tile_pool(name="psum_pair", bufs=1, space="PSUM")
            cx.psum = [ppp.tile([128, 2 * N], F32, name=f"pp{q}")
                       for q in range(3)]
            cx.upsum = [ppp.tile([128, N], F32, name=f"up{q}")
                        for q in range(2)]
            cx.up_rr = 0

            # zero pairwise psum once so the sigmoid's unused partitions
            # stay finite
            zlhs = cpool.tile([128, 128], BF16, name="zlhs")
            zrhs = cpool.tile([128, N], BF16, name="zrhs")
            nc.vector.memset(zlhs[:, :], 0.0)
            nc.vector.memset(zrhs[:, :], 0.0)
            for q in range(3):
                for half in range(2):
                    nc.tensor.matmul(
                        cx.psum[q][:, 512 * half : 512 * (half + 1)],
                        lhsT=zlhs[:, :], rhs=zrhs[:, 0:512],
                        start=True, stop=True)

            # prologue: full upstream for rep 0
            st = {}
            for s in ("xt", "h", "f", "c", "a"):
                _upstream_stage(nc, cx, st, s)

            for r in range(reps):
                nxt = {} if r + 1 < reps else None
                _rep_body(nc, cx, st["cts"], st["ats"], nxt)
                if nxt is not None:
                    st = nxt

            ppp.release()

    return nc


_NC_CACHE = {}


def _get_nc(reps: int = 1):
    if reps not in _NC_CACHE:
        _NC_CACHE[reps] = build_nc(reps)
    return _NC_CACHE[reps]


def _pack_w(wT):
    """[256, 256] pre-transposed weight -> [128, 512] fp8 DoubleRow tile."""
    fp8 = ml_dtypes.float8_e4m3
    w8 = np.concatenate([wT[0:128, :], wT[128:256, :]], axis=1)
    return np.ascontiguousarray(w8 * WS).astype(fp8)


def make_in_maps(step_sequence, step_mask, W_gc1, b_gc1, W_gc2, b_gc2,
                 W_ep1, b_ep1, w_ep2, b_ep2):
    x = np.ascontiguousarray(np.asarray(step_sequence, dtype=np.float32))
    W_gc1 = np.asarray(W_gc1, np.float32)
    W_gc2 = np.asarray(W_gc2, np.float32)
    W_ep1 = np.asarray(W_ep1, np.float32)
    b_gc1 = np.asarray(b_gc1, np.float32)
    b_gc2 = np.asarray(b_gc2, np.float32)
    b_ep1 = np.asarray(b_ep1, np.float32)
    w_ep2 = np.asarray(w_ep2, np.float32)
    b_ep2v = np.float32(np.asarray(b_ep2))

    bf16 = ml_dtypes.bfloat16
    fp8 = ml_dtypes.float8_e4m3
    w1t = _pack_w(np.ascontiguousarray(W_gc1.T))
    w2t = _pack_w(np.ascontiguousarray(W_gc2.T))
    wat = _pack_w(np.ascontiguousarray(W_ep1[:, :H].T) * E_SCALE)
    wbt = _pack_w(np.ascontiguousarray(W_ep1[:, H:].T) * E_SCALE)
    b1m = np.ascontiguousarray(b_gc1.reshape(2, 128).T)
    b2m = np.ascontiguousarray(b_gc2.reshape(2, 128).T)
    bep1m = np.ascontiguousarray(b_ep1.reshape(2, 128).T * E_SCALE)
    wep2m = np.ascontiguousarray(
        (w_ep2 * W2_SCALE).reshape(2, 128).T).astype(bf16)
    wep2f8m = np.zeros((128, 2048), fp8)
    for u in range(4):
        # u-variant [p, t(2), m(256)]: w at column m = 32u, zero elsewhere
        wep2f8m[:, 512 * u + 32 * u] = (w_ep2[:128] * W2_SCALE).astype(fp8)
        wep2f8m[:, 512 * u + 256 + 32 * u] = (
            w_ep2[128:] * W2_SCALE).astype(fp8)
    bep2m = np.full((128, 1), b_ep2v, np.float32)

    in_maps = []
    for d in range(NCORES):
        b, ph = divmod(d, 4)
        my_i = np.arange(ph, N, 4)
        xT = x[b].T  # [H, N]
        xTmy = np.ascontiguousarray(x[b][my_i].T)  # [H, R]
        x640 = np.concatenate([xT, xTmy], axis=1)  # [256, 640]
        xt8 = np.ascontiguousarray(
            np.concatenate([x640[0:128, :], x640[128:256, :]], axis=1)
        ).astype(fp8)  # [128, 1280]
        in_maps.append({
            "xt": xt8, "w1t": w1t, "w2t": w2t, "wat": wat, "wbt": wbt,
            "b1": b1m, "b2": b2m, "bep1": bep1m, "wep2": wep2m,
            "wep2f8": wep2f8m, "bep2": bep2m,
        })
    return in_maps


_MASK_CACHE = {}


def _tril_mask():
    if "m" not in _MASK_CACHE:
        _MASK_CACHE["m"] = np.tril(np.ones((N, N), np.float32), k=-1)
    return _MASK_CACHE["m"]


def gather_output(results):
    out = np.zeros((B, N, N), np.float32)
    for d in range(NCORES):
        b, ph = divmod(d, 4)
        dev = results[d]["y"]  # [R, N]
        for g in range(16):
            J = min(N, 32 * (g + 1))
            ks = np.arange(8 * g, 8 * (g + 1))
            out[b, 4 * ks + ph, :J] = dev[8 * g : 8 * (g + 1), :J]
    out *= _tril_mask()[None, :, :]
    return out


def kernel(**inputs) -> np.ndarray:
    nc = _get_nc()
    in_maps = make_in_maps(**inputs)
    res = run_bass_kernel_spmd(nc, in_maps, core_ids=list(range(NCORES)))
    return gather_output(res.results)

